# revision 4
# baseline (speedup 1.0000x reference)
"""Multi-head self-attention (dense transformer block) on 8 TRN2 NeuronCores.

Data-parallel over batch: 8 batch items -> 8 cores, one image each, zero
collectives.  Per core the kernel computes, for x_b in [C=512, S=1024] layout
(channels x positions, which is x[b].reshape(C, H*W) -- i.e. xs^T):

  QT = Wq^T @ x_b            [nh*dk, S]   (heads on partition tiles)
  KT = Wk^T @ x_b            [nh*dk, S]
  V  = x_b^T @ Wv            [S, nh*dv]   (positions on partitions), with an
                                          appended ones-column per head
  per head h:
    st  = K_h @ Q_h^T        [S_k, S_q]   (k-positions on partitions)
    est = exp(st / 8)                     (ScalarE; no max-subtraction --
                                           scores are ~N(0,1), max ~5)
    pv  = [V_h | 1]^T @ est  [dv+1, S_q]  row dv holds sum_k est = softmax
                                           denominator (free on TensorE)
    attnT_h = pv[:dv] * (1/pv[dv])        per-q normalization
  outT = Wo^T @ attnT + x_b  [C, S]       residual; exactly the output layout

All matmuls run as float32r (fp32 storage, single-pass reduced-precision PE
mode: 1 cycle/row at N=512 vs 4 for fp32).
"""

import numpy as np

B = 8
C = 512
S = 1024
NH = 8
D = 64
P = 128
KO = C // P  # 4 partition tiles over the channel/contract dim
SO = S // P  # 8 partition tiles over positions
NQ = S // 512  # 2 free-dim chunks of 512 (fp32 moving-operand max)

_GRAPH_CACHE = {}


def _r(ap):
    """View an fp32 AP as float32r for full-rate PE matmuls."""
    import concourse.mybir as mybir

    return ap.bitcast(mybir.dt.float32r)


def _build_graph(with_bias: bool):
    import concourse.bass as bass
    import concourse.tile as tile
    from concourse import bacc, mybir
    from contextlib import ExitStack

    F32 = mybir.dt.float32
    Exp = mybir.ActivationFunctionType.Exp
    ADD = mybir.AluOpType.add
    MUL = mybir.AluOpType.mult

    nc = bacc.Bacc("TRN2", target_bir_lowering=False, debug=False, num_devices=B)

    x = nc.declare_dram_parameter("x", [C, S], F32, isOutput=False)
    wq = nc.declare_dram_parameter("Wq", [C, NH * D], F32, isOutput=False)
    wk = nc.declare_dram_parameter("Wk", [C, NH * D], F32, isOutput=False)
    wv = nc.declare_dram_parameter("Wv", [C, NH * D], F32, isOutput=False)
    wo = nc.declare_dram_parameter("Wo", [NH * D, C], F32, isOutput=False)
    if with_bias:
        bq = nc.declare_dram_parameter("bq", [NH * D], F32, isOutput=False)
        bk = nc.declare_dram_parameter("bk", [NH * D], F32, isOutput=False)
        bv = nc.declare_dram_parameter("bv", [NH * D], F32, isOutput=False)
        bo = nc.declare_dram_parameter("bo", [C], F32, isOutput=False)
    out = nc.declare_dram_parameter("out", [C, S], F32, isOutput=True)

    with ExitStack() as ctx:
        tc = ctx.enter_context(tile.TileContext(nc))
        singles = ctx.enter_context(tc.tile_pool(name="singles", bufs=1))
        mm_ps = ctx.enter_context(tc.tile_pool(name="mm_ps", bufs=2, space="PSUM"))
        st_ps = ctx.enter_context(tc.tile_pool(name="st_ps", bufs=2, space="PSUM"))
        pv_ps = ctx.enter_context(tc.tile_pool(name="pv_ps", bufs=1, space="PSUM"))
        est_po = ctx.enter_context(tc.tile_pool(name="est_po", bufs=10))
        out_po = ctx.enter_context(tc.tile_pool(name="out_po", bufs=3))
        rr_po = ctx.enter_context(tc.tile_pool(name="rr_po", bufs=2))

        xb = singles.tile([P, KO, S], F32, tag="xb")
        wq_sb = singles.tile([P, KO, NH * D], F32, tag="wq")
        wk_sb = singles.tile([P, KO, NH * D], F32, tag="wk")
        wv_sb = singles.tile([P, KO, NH * D], F32, tag="wv")
        wo_sb = singles.tile([P, KO, C], F32, tag="wo")
        qt_sb = singles.tile([P, KO, S], F32, tag="qt")
        kt_sb = singles.tile([P, KO, S], F32, tag="kt")
        v_sb = singles.tile([P, SO, NH, D + 1], F32, tag="v")
        at_sb = singles.tile([P, KO, S], F32, tag="at")

        nc.sync.dma_start(out=_r(wq_sb[:]), in_=_r(wq.rearrange("(ko p) n -> p ko n", p=P)))
        nc.sync.dma_start(out=_r(xb[:]), in_=_r(x.rearrange("(ko p) s -> p ko s", p=P)))
        nc.sync.dma_start(out=_r(wk_sb[:]), in_=_r(wk.rearrange("(ko p) n -> p ko n", p=P)))
        nc.sync.dma_start(out=_r(wv_sb[:]), in_=_r(wv.rearrange("(ko p) n -> p ko n", p=P)))
        nc.sync.dma_start(out=_r(wo_sb[:]), in_=_r(wo.rearrange("(ko p) n -> p ko n", p=P)))
        ones_c = singles.tile([P, 1], F32, tag="ones")
        nc.vector.memset(ones_c[:], 1.0)
        nc.vector.tensor_copy(
            out=_r(v_sb[:, :, :, D : D + 1]),
            in_=ones_c[:].to_broadcast((P, SO, NH, 1)),
        )

        if with_bias:
            # bq/bk land on partitions (per output channel); bv along free.
            bq_sb = singles.tile([P, KO, 1], F32, tag="bq")
            bk_sb = singles.tile([P, KO, 1], F32, tag="bk")
            nc.sync.dma_start(out=bq_sb[:, :, 0], in_=bq.rearrange("(ko p) -> p ko", p=P))
            nc.sync.dma_start(out=bk_sb[:, :, 0], in_=bk.rearrange("(ko p) -> p ko", p=P))
            bv_rep = singles.tile([P, NH * D], F32, tag="bv")
            nc.sync.dma_start(
                out=bv_rep[:],
                in_=bass.AP(tensor=bv.tensor, offset=bv.offset, ap=[[0, P], [1, NH * D]]),
            )
            # xbo = x_b + bo (per-channel => per-partition scalar)
            bo_sb = singles.tile([P, KO, 1], F32, tag="bo")
            nc.sync.dma_start(out=bo_sb[:, :, 0], in_=bo.rearrange("(ko p) -> p ko", p=P))
            xbo = singles.tile([P, KO, S], F32, tag="xbo")
            for ko in range(KO):
                nc.vector.tensor_scalar_add(
                    out=xbo[:, ko, :], in0=xb[:, ko, :], scalar1=bo_sb[:, ko]
                )
            resid = xbo
        else:
            resid = xb

        # ---- QT / KT projections: psum[mo, qc] = sum_ko Wq[ko,mo]^T x[ko,qc]
        for w_sb, t_sb, b_sb in ((wq_sb, qt_sb, "bq"), (wk_sb, kt_sb, "bk")):
            for mo in range(KO):
                for qc in range(NQ):
                    ps = mm_ps.tile([P, 512], F32, tag="mmps")
                    for ko in range(KO):
                        nc.tensor.matmul(
                            ps[:],
                            _r(w_sb[:, ko, mo * P : (mo + 1) * P]),
                            _r(xb[:, ko, qc * 512 : (qc + 1) * 512]),
                            start=(ko == 0),
                            stop=(ko == KO - 1),
                        )
                    dst = _r(t_sb[:, mo, qc * 512 : (qc + 1) * 512])
                    if with_bias:
                        bias_t = bq_sb if b_sb == "bq" else bk_sb
                        nc.vector.tensor_scalar_add(
                            out=dst, in0=ps[:], scalar1=bias_t[:, mo]
                        )
                    else:
                        nc.vector.tensor_copy(out=dst, in_=ps[:])

        # ---- V projection: psum[so] = sum_ko x[ko,so]^T Wv[ko,:]  -> [s, nh*dv]
        for so in range(SO):
            ps = mm_ps.tile([P, 512], F32, tag="mmps")
            for ko in range(KO):
                nc.tensor.matmul(
                    ps[:],
                    _r(xb[:, ko, so * P : (so + 1) * P]),
                    _r(wv_sb[:, ko, :]),
                    start=(ko == 0),
                    stop=(ko == KO - 1),
                )
            dst = _r(v_sb[:, so, :, 0:D])  # [P, NH, D] strided (stride D+1)
            src = ps[:].rearrange("p (h d) -> p h d", h=NH)
            if with_bias:
                nc.vector.tensor_tensor(
                    dst, src, bv_rep[:].rearrange("p (h d) -> p h d", h=NH), ADD
                )
            else:
                nc.vector.tensor_copy(out=dst, in_=src)

        # ---- attention per head
        for h in range(NH):
            hp = h // 2
            hr = (h % 2) * D
            pv = pv_ps.tile([D + 1, S], F32, tag="pvps")
            for ki in range(SO):
                st = st_ps.tile([P, S], F32, tag="stps")
                for qc in range(NQ):
                    nc.tensor.matmul(
                        st[:, qc * 512 : (qc + 1) * 512],
                        _r(kt_sb[hr : hr + D, hp, ki * P : (ki + 1) * P]),
                        _r(qt_sb[hr : hr + D, hp, qc * 512 : (qc + 1) * 512]),
                        start=True,
                        stop=True,
                    )
                est = est_po.tile([P, S], F32, tag="est")
                nc.scalar.activation(out=_r(est[:]), in_=st[:], func=Exp, scale=1.0 / 8.0)
                for qc in range(NQ):
                    nc.tensor.matmul(
                        pv[:, qc * 512 : (qc + 1) * 512],
                        _r(v_sb[:, ki, h, :]),
                        _r(est[:, qc * 512 : (qc + 1) * 512]),
                        start=(ki == 0),
                        stop=(ki == SO - 1),
                    )
            rrow = rr_po.tile([1, S], F32, tag="rrow")
            nc.vector.reciprocal(out=rrow[:], in_=pv[D : D + 1, :])
            rrep = rr_po.tile([D, S], F32, tag="rrep")
            nc.gpsimd.partition_broadcast(rrep[:], rrow[0:1, :])
            nc.vector.tensor_tensor(_r(at_sb[hr : hr + D, hp, :]), pv[0:D, :], rrep[:], MUL)

        # ---- output projection + residual: outT[mo, qc] = Wo^T attnT + x_b
        out_r = out.rearrange("(mo p) s -> p mo s", p=P)
        for mo in range(KO):
            for qc in range(NQ):
                ps = mm_ps.tile([P, 512], F32, tag="mmps")
                for ko in range(KO):
                    nc.tensor.matmul(
                        ps[:],
                        _r(wo_sb[:, ko, mo * P : (mo + 1) * P]),
                        _r(at_sb[:, ko, qc * 512 : (qc + 1) * 512]),
                        start=(ko == 0),
                        stop=(ko == KO - 1),
                    )
                ot = out_po.tile([P, 512], F32, tag="ot")
                nc.vector.tensor_add(
                    out=ot[:], in0=ps[:], in1=resid[:, mo, qc * 512 : (qc + 1) * 512]
                )
                nc.sync.dma_start(out=out_r[:, mo, qc * 512 : (qc + 1) * 512], in_=ot[:])

    nc.compile()
    return nc


def _get_graph(with_bias: bool):
    key = bool(with_bias)
    if key not in _GRAPH_CACHE:
        _GRAPH_CACHE[key] = _build_graph(key)
    return _GRAPH_CACHE[key]


def _make_in_maps(inputs, with_bias: bool):
    x = np.ascontiguousarray(np.asarray(inputs["x"], dtype=np.float32))
    assert x.shape == (B, C, 32, 32), x.shape
    xf = x.reshape(B, C, S)
    ws = {
        k: np.ascontiguousarray(np.asarray(inputs[k], dtype=np.float32))
        for k in ("Wq", "Wk", "Wv", "Wo")
    }
    maps = []
    for b in range(B):
        m = {"x": np.ascontiguousarray(xf[b])}
        m.update(ws)
        if with_bias:
            for k in ("bq", "bk", "bv", "bo"):
                m[k] = np.ascontiguousarray(np.asarray(inputs[k], dtype=np.float32))
        maps.append(m)
    return maps


def _run(inputs, **spmd_kwargs):
    from concourse.bass_utils import run_bass_kernel_spmd

    nh = int(np.asarray(inputs.get("num_heads", NH)))
    assert nh == NH, f"kernel hardcodes num_heads={NH}, got {nh}"
    with_bias = any(
        np.any(np.asarray(inputs[k])) for k in ("bq", "bk", "bv", "bo") if k in inputs
    )
    nc = _get_graph(with_bias)
    in_maps = _make_in_maps(inputs, with_bias)
    res = run_bass_kernel_spmd(nc, in_maps, core_ids=list(range(B)), **spmd_kwargs)
    outs = np.stack([res.results[b]["out"] for b in range(B)])  # [B, C, S]
    return outs.reshape(B, C, 32, 32).astype(np.float32), res


def kernel(**inputs):
    out, _ = _run(inputs)
    return out


# revision 7
# speedup vs baseline: 1.0585x; 1.0585x over previous
"""Multi-head self-attention (dense transformer block) on 8 TRN2 NeuronCores.

Data-parallel over batch: 8 batch items -> 8 cores, one image each, zero
collectives.  Per core the kernel computes, for x_b in [C=512, S=1024] layout
(channels x positions, which is x[b].reshape(C, H*W) -- i.e. xs^T):

  QT = Wq^T @ x_b            [nh*dk, S]   (heads on partition tiles)
  KT = Wk^T @ x_b            [nh*dk, S]
  V  = x_b^T @ Wv            [S, nh*dv]   (positions on partitions), with an
                                          appended ones-column per head
  per head h:
    st  = K_h @ Q_h^T        [S_k, S_q]   (k-positions on partitions)
    est = exp(st / 8)                     (ScalarE; no max-subtraction --
                                           scores are ~N(0,1), max ~5)
    pv  = [V_h | 1]^T @ est  [dv+1, S_q]  row dv holds sum_k est = softmax
                                           denominator (free on TensorE)
    attnT_h = pv[:dv] * (1/pv[dv])        per-q normalization
  outT = Wo^T @ attnT + x_b  [C, S]       residual; exactly the output layout

All matmuls run as float32r (fp32 storage, single-pass reduced-precision PE
mode: 1 cycle/row at N=512 vs 4 for fp32).
"""

import numpy as np

B = 8
C = 512
S = 1024
NH = 8
D = 64
P = 128
KO = C // P  # 4 partition tiles over the channel/contract dim
SO = S // P  # 8 partition tiles over positions
NQ = S // 512  # 2 free-dim chunks of 512 (fp32 moving-operand max)

_GRAPH_CACHE = {}


def _r(ap):
    """View an fp32 AP as float32r for full-rate PE matmuls."""
    import concourse.mybir as mybir

    return ap.bitcast(mybir.dt.float32r)


def _build_graph(with_bias: bool):
    import concourse.bass as bass
    import concourse.tile as tile
    from concourse import bacc, mybir
    from contextlib import ExitStack

    F32 = mybir.dt.float32
    Exp = mybir.ActivationFunctionType.Exp
    ADD = mybir.AluOpType.add
    MUL = mybir.AluOpType.mult

    nc = bacc.Bacc("TRN2", target_bir_lowering=False, debug=False, num_devices=B)

    x = nc.declare_dram_parameter("x", [C, S], F32, isOutput=False)
    wq = nc.declare_dram_parameter("Wq", [C, NH * D], F32, isOutput=False)
    wk = nc.declare_dram_parameter("Wk", [C, NH * D], F32, isOutput=False)
    wv = nc.declare_dram_parameter("Wv", [C, NH * D], F32, isOutput=False)
    wo = nc.declare_dram_parameter("Wo", [NH * D, C], F32, isOutput=False)
    if with_bias:
        bq = nc.declare_dram_parameter("bq", [NH * D], F32, isOutput=False)
        bk = nc.declare_dram_parameter("bk", [NH * D], F32, isOutput=False)
        bv = nc.declare_dram_parameter("bv", [NH * D], F32, isOutput=False)
        bo = nc.declare_dram_parameter("bo", [C], F32, isOutput=False)
    out = nc.declare_dram_parameter("out", [C, S], F32, isOutput=True)

    with ExitStack() as ctx:
        tc = ctx.enter_context(tile.TileContext(nc))
        singles = ctx.enter_context(tc.tile_pool(name="singles", bufs=1))
        est_po = ctx.enter_context(tc.tile_pool(name="est_po", bufs=10))
        out_po = ctx.enter_context(tc.tile_pool(name="out_po", bufs=3))
        rr_po = ctx.enter_context(tc.tile_pool(name="rr_po", bufs=2))

        xb = singles.tile([P, KO, S], F32, tag="xb")
        wq_sb = singles.tile([P, KO, NH * D], F32, tag="wq")
        wk_sb = singles.tile([P, KO, NH * D], F32, tag="wk")
        wv_sb = singles.tile([P, KO, NH * D], F32, tag="wv")
        wo_sb = singles.tile([P, KO, C], F32, tag="wo")
        qt_sb = singles.tile([P, KO, S], F32, tag="qt")
        kt_sb = singles.tile([P, KO, S], F32, tag="kt")
        v_sb = singles.tile([P, SO, NH, D + 1], F32, tag="v")
        at_sb = singles.tile([P, KO, S], F32, tag="at")

        nc.sync.dma_start(out=_r(wq_sb[:]), in_=_r(wq.rearrange("(ko p) n -> p ko n", p=P)))
        nc.sync.dma_start(out=_r(xb[:]), in_=_r(x.rearrange("(ko p) s -> p ko s", p=P)))
        nc.sync.dma_start(out=_r(wk_sb[:]), in_=_r(wk.rearrange("(ko p) n -> p ko n", p=P)))
        nc.sync.dma_start(out=_r(wv_sb[:]), in_=_r(wv.rearrange("(ko p) n -> p ko n", p=P)))
        nc.sync.dma_start(out=_r(wo_sb[:]), in_=_r(wo.rearrange("(ko p) n -> p ko n", p=P)))
        ones_c = singles.tile([P, 1], F32, tag="ones")
        nc.vector.memset(ones_c[:], 1.0)
        nc.vector.tensor_copy(
            out=_r(v_sb[:, :, :, D : D + 1]),
            in_=ones_c[:].to_broadcast((P, SO, NH, 1)),
        )

        if with_bias:
            # bq/bk land on partitions (per output channel); bv along free.
            bq_sb = singles.tile([P, KO, 1], F32, tag="bq")
            bk_sb = singles.tile([P, KO, 1], F32, tag="bk")
            nc.sync.dma_start(out=bq_sb[:, :, 0], in_=bq.rearrange("(ko p) -> p ko", p=P))
            nc.sync.dma_start(out=bk_sb[:, :, 0], in_=bk.rearrange("(ko p) -> p ko", p=P))
            bv_rep = singles.tile([P, NH * D], F32, tag="bv")
            nc.sync.dma_start(
                out=bv_rep[:],
                in_=bass.AP(tensor=bv.tensor, offset=bv.offset, ap=[[0, P], [1, NH * D]]),
            )
            # xbo = x_b + bo (per-channel => per-partition scalar)
            bo_sb = singles.tile([P, KO, 1], F32, tag="bo")
            nc.sync.dma_start(out=bo_sb[:, :, 0], in_=bo.rearrange("(ko p) -> p ko", p=P))
            xbo = singles.tile([P, KO, S], F32, tag="xbo")
            for ko in range(KO):
                nc.vector.tensor_scalar_add(
                    out=xbo[:, ko, :], in0=xb[:, ko, :], scalar1=bo_sb[:, ko]
                )
            resid = xbo
        else:
            resid = xb

        # ---- QT / KT projections: psum[mo, qc] = sum_ko Wq[ko,mo]^T x[ko,qc]
        mm_ctx = tc.tile_pool(name="mm_ps", bufs=2, space="PSUM")
        mm_ps = mm_ctx.__enter__()
        for w_sb, t_sb, b_sb in ((wq_sb, qt_sb, "bq"), (wk_sb, kt_sb, "bk")):
            for mo in range(KO):
                for qc in range(NQ):
                    ps = mm_ps.tile([P, 512], F32, tag="mmps")
                    for ko in range(KO):
                        nc.tensor.matmul(
                            ps[:],
                            _r(w_sb[:, ko, mo * P : (mo + 1) * P]),
                            _r(xb[:, ko, qc * 512 : (qc + 1) * 512]),
                            start=(ko == 0),
                            stop=(ko == KO - 1),
                        )
                    dst = _r(t_sb[:, mo, qc * 512 : (qc + 1) * 512])
                    if with_bias:
                        bias_t = bq_sb if b_sb == "bq" else bk_sb
                        nc.vector.tensor_scalar_add(
                            out=dst, in0=ps[:], scalar1=bias_t[:, mo]
                        )
                    else:
                        nc.vector.tensor_copy(out=dst, in_=ps[:])

        # ---- V projection: psum[so] = sum_ko x[ko,so]^T Wv[ko,:]  -> [s, nh*dv]
        for so in range(SO):
            ps = mm_ps.tile([P, 512], F32, tag="mmps")
            for ko in range(KO):
                nc.tensor.matmul(
                    ps[:],
                    _r(xb[:, ko, so * P : (so + 1) * P]),
                    _r(wv_sb[:, ko, :]),
                    start=(ko == 0),
                    stop=(ko == KO - 1),
                )
            dst = _r(v_sb[:, so, :, 0:D])  # [P, NH, D] strided (stride D+1)
            src = ps[:].rearrange("p (h d) -> p h d", h=NH)
            if with_bias:
                nc.vector.tensor_tensor(
                    dst, src, bv_rep[:].rearrange("p (h d) -> p h d", h=NH), ADD
                )
            else:
                nc.vector.tensor_copy(out=dst, in_=src)

        mm_ctx.__exit__(None, None, None)

        # ---- attention per head
        st_ctx = tc.tile_pool(name="st_ps", bufs=2, space="PSUM")
        pv_ctx = tc.tile_pool(name="pv_ps", bufs=2, space="PSUM")
        st_ps = st_ctx.__enter__()
        pv_ps = pv_ctx.__enter__()
        for h in range(NH):
            hp = h // 2
            hr = (h % 2) * D
            pv = pv_ps.tile([D + 1, S], F32, tag="pvps")
            for ki in range(SO):
                st = st_ps.tile([P, S], F32, tag="stps")
                for qc in range(NQ):
                    nc.tensor.matmul(
                        st[:, qc * 512 : (qc + 1) * 512],
                        _r(kt_sb[hr : hr + D, hp, ki * P : (ki + 1) * P]),
                        _r(qt_sb[hr : hr + D, hp, qc * 512 : (qc + 1) * 512]),
                        start=True,
                        stop=True,
                    )
                est = est_po.tile([P, S], F32, tag="est")
                nc.scalar.activation(out=_r(est[:]), in_=st[:], func=Exp, scale=1.0 / 8.0)
                for qc in range(NQ):
                    nc.tensor.matmul(
                        pv[:, qc * 512 : (qc + 1) * 512],
                        _r(v_sb[:, ki, h, :]),
                        _r(est[:, qc * 512 : (qc + 1) * 512]),
                        start=(ki == 0),
                        stop=(ki == SO - 1),
                    )
            rrow = rr_po.tile([1, S], F32, tag="rrow")
            nc.vector.reciprocal(out=rrow[:], in_=pv[D : D + 1, :])
            rrep = rr_po.tile([D, S], F32, tag="rrep")
            nc.gpsimd.partition_broadcast(rrep[:], rrow[0:1, :])
            nc.vector.tensor_tensor(_r(at_sb[hr : hr + D, hp, :]), pv[0:D, :], rrep[:], MUL)

        pv_ctx.__exit__(None, None, None)
        st_ctx.__exit__(None, None, None)

        # ---- output projection + residual: outT[mo, qc] = Wo^T attnT + x_b
        mo_ctx = tc.tile_pool(name="mo_ps", bufs=2, space="PSUM")
        mm_ps = mo_ctx.__enter__()
        out_r = out.rearrange("(mo p) s -> p mo s", p=P)
        for mo in range(KO):
            for qc in range(NQ):
                ps = mm_ps.tile([P, 512], F32, tag="mmps")
                for ko in range(KO):
                    nc.tensor.matmul(
                        ps[:],
                        _r(wo_sb[:, ko, mo * P : (mo + 1) * P]),
                        _r(at_sb[:, ko, qc * 512 : (qc + 1) * 512]),
                        start=(ko == 0),
                        stop=(ko == KO - 1),
                    )
                ot = out_po.tile([P, 512], F32, tag="ot")
                nc.vector.tensor_add(
                    out=ot[:], in0=ps[:], in1=resid[:, mo, qc * 512 : (qc + 1) * 512]
                )
                nc.sync.dma_start(out=out_r[:, mo, qc * 512 : (qc + 1) * 512], in_=ot[:])
        mo_ctx.__exit__(None, None, None)

    nc.compile()
    return nc


def _get_graph(with_bias: bool):
    key = bool(with_bias)
    if key not in _GRAPH_CACHE:
        _GRAPH_CACHE[key] = _build_graph(key)
    return _GRAPH_CACHE[key]


def _make_in_maps(inputs, with_bias: bool):
    x = np.ascontiguousarray(np.asarray(inputs["x"], dtype=np.float32))
    assert x.shape == (B, C, 32, 32), x.shape
    xf = x.reshape(B, C, S)
    ws = {
        k: np.ascontiguousarray(np.asarray(inputs[k], dtype=np.float32))
        for k in ("Wq", "Wk", "Wv", "Wo")
    }
    maps = []
    for b in range(B):
        m = {"x": np.ascontiguousarray(xf[b])}
        m.update(ws)
        if with_bias:
            for k in ("bq", "bk", "bv", "bo"):
                m[k] = np.ascontiguousarray(np.asarray(inputs[k], dtype=np.float32))
        maps.append(m)
    return maps


def _run(inputs, **spmd_kwargs):
    from concourse.bass_utils import run_bass_kernel_spmd

    nh = int(np.asarray(inputs.get("num_heads", NH)))
    assert nh == NH, f"kernel hardcodes num_heads={NH}, got {nh}"
    with_bias = any(
        np.any(np.asarray(inputs[k])) for k in ("bq", "bk", "bv", "bo") if k in inputs
    )
    nc = _get_graph(with_bias)
    in_maps = _make_in_maps(inputs, with_bias)
    res = run_bass_kernel_spmd(nc, in_maps, core_ids=list(range(B)), **spmd_kwargs)
    outs = np.stack([res.results[b]["out"] for b in range(B)])  # [B, C, S]
    return outs.reshape(B, C, 32, 32).astype(np.float32), res


def kernel(**inputs):
    out, _ = _run(inputs)
    return out


# revision 8
# speedup vs baseline: 1.0627x; 1.0041x over previous
"""Multi-head self-attention (dense transformer block) on 8 TRN2 NeuronCores.

Data-parallel over batch: 8 batch items -> 8 cores, one image each, zero
collectives.  Per core the kernel computes, for x_b in [C=512, S=1024] layout
(channels x positions, which is x[b].reshape(C, H*W) -- i.e. xs^T):

  QT = Wq^T @ x_b            [nh*dk, S]   (heads on partition tiles)
  KT = Wk^T @ x_b            [nh*dk, S]
  V  = x_b^T @ Wv            [S, nh*dv]   (positions on partitions), with an
                                          appended ones-column per head
  per head h:
    st  = K_h @ Q_h^T        [S_k, S_q]   (k-positions on partitions)
    est = exp(st / 8)                     (ScalarE; no max-subtraction --
                                           scores are ~N(0,1), max ~5)
    pv  = [V_h | 1]^T @ est  [dv+1, S_q]  row dv holds sum_k est = softmax
                                           denominator (free on TensorE)
    attnT_h = pv[:dv] * (1/pv[dv])        per-q normalization
  outT = Wo^T @ attnT + x_b  [C, S]       residual; exactly the output layout

All matmuls run as float32r (fp32 storage, single-pass reduced-precision PE
mode: 1 cycle/row at N=512 vs 4 for fp32).
"""

import numpy as np

B = 8
C = 512
S = 1024
NH = 8
D = 64
P = 128
KO = C // P  # 4 partition tiles over the channel/contract dim
SO = S // P  # 8 partition tiles over positions
NQ = S // 512  # 2 free-dim chunks of 512 (fp32 moving-operand max)

_GRAPH_CACHE = {}


def _r(ap):
    """View an fp32 AP as float32r for full-rate PE matmuls."""
    import concourse.mybir as mybir

    return ap.bitcast(mybir.dt.float32r)


def _build_graph(with_bias: bool):
    import concourse.bass as bass
    import concourse.tile as tile
    from concourse import bacc, mybir
    from contextlib import ExitStack

    F32 = mybir.dt.float32
    Exp = mybir.ActivationFunctionType.Exp
    ADD = mybir.AluOpType.add
    MUL = mybir.AluOpType.mult

    nc = bacc.Bacc("TRN2", target_bir_lowering=False, debug=False, num_devices=B)

    x = nc.declare_dram_parameter("x", [C, S], F32, isOutput=False)
    wq = nc.declare_dram_parameter("Wq", [C, NH * D], F32, isOutput=False)
    wk = nc.declare_dram_parameter("Wk", [C, NH * D], F32, isOutput=False)
    wv = nc.declare_dram_parameter("Wv", [C, NH * D], F32, isOutput=False)
    wo = nc.declare_dram_parameter("Wo", [NH * D, C], F32, isOutput=False)
    if with_bias:
        bq = nc.declare_dram_parameter("bq", [NH * D], F32, isOutput=False)
        bk = nc.declare_dram_parameter("bk", [NH * D], F32, isOutput=False)
        bv = nc.declare_dram_parameter("bv", [NH * D], F32, isOutput=False)
        bo = nc.declare_dram_parameter("bo", [C], F32, isOutput=False)
    out = nc.declare_dram_parameter("out", [C, S], F32, isOutput=True)

    with ExitStack() as ctx:
        tc = ctx.enter_context(tile.TileContext(nc))
        singles = ctx.enter_context(tc.tile_pool(name="singles", bufs=1))
        est_po = ctx.enter_context(tc.tile_pool(name="est_po", bufs=10))
        out_po = ctx.enter_context(tc.tile_pool(name="out_po", bufs=3))
        rr_po = ctx.enter_context(tc.tile_pool(name="rr_po", bufs=2))

        xb = singles.tile([P, KO, S], F32, tag="xb")
        wq_sb = singles.tile([P, KO, NH * D], F32, tag="wq")
        wk_sb = singles.tile([P, KO, NH * D], F32, tag="wk")
        wv_sb = singles.tile([P, KO, NH * D], F32, tag="wv")
        wo_sb = singles.tile([P, KO, C], F32, tag="wo")
        qt_sb = singles.tile([P, KO, S], F32, tag="qt")
        kt_sb = singles.tile([P, KO, S], F32, tag="kt")
        v_sb = singles.tile([P, SO, NH, D + 1], F32, tag="v")
        at_sb = singles.tile([P, KO, S], F32, tag="at")

        nc.sync.dma_start(out=_r(wq_sb[:]), in_=_r(wq.rearrange("(ko p) n -> p ko n", p=P)))
        nc.sync.dma_start(out=_r(xb[:]), in_=_r(x.rearrange("(ko p) s -> p ko s", p=P)))
        nc.sync.dma_start(out=_r(wk_sb[:]), in_=_r(wk.rearrange("(ko p) n -> p ko n", p=P)))
        nc.sync.dma_start(out=_r(wv_sb[:]), in_=_r(wv.rearrange("(ko p) n -> p ko n", p=P)))
        nc.sync.dma_start(out=_r(wo_sb[:]), in_=_r(wo.rearrange("(ko p) n -> p ko n", p=P)))
        ones_c = singles.tile([P, 1], F32, tag="ones")
        nc.vector.memset(ones_c[:], 1.0)
        nc.vector.tensor_copy(
            out=_r(v_sb[:, :, :, D : D + 1]),
            in_=ones_c[:].to_broadcast((P, SO, NH, 1)),
        )

        if with_bias:
            # bq/bk land on partitions (per output channel); bv along free.
            bq_sb = singles.tile([P, KO, 1], F32, tag="bq")
            bk_sb = singles.tile([P, KO, 1], F32, tag="bk")
            nc.sync.dma_start(out=bq_sb[:, :, 0], in_=bq.rearrange("(ko p) -> p ko", p=P))
            nc.sync.dma_start(out=bk_sb[:, :, 0], in_=bk.rearrange("(ko p) -> p ko", p=P))
            bv_rep = singles.tile([P, NH * D], F32, tag="bv")
            nc.sync.dma_start(
                out=bv_rep[:],
                in_=bass.AP(tensor=bv.tensor, offset=bv.offset, ap=[[0, P], [1, NH * D]]),
            )
            # xbo = x_b + bo (per-channel => per-partition scalar)
            bo_sb = singles.tile([P, KO, 1], F32, tag="bo")
            nc.sync.dma_start(out=bo_sb[:, :, 0], in_=bo.rearrange("(ko p) -> p ko", p=P))
            xbo = singles.tile([P, KO, S], F32, tag="xbo")
            for ko in range(KO):
                nc.vector.tensor_scalar_add(
                    out=xbo[:, ko, :], in0=xb[:, ko, :], scalar1=bo_sb[:, ko]
                )
            resid = xbo
        else:
            resid = xb

        # ---- QT / KT projections: psum[mo, qc] = sum_ko Wq[ko,mo]^T x[ko,qc]
        mm_ctx = tc.tile_pool(name="mm_ps", bufs=2, space="PSUM")
        mm_ps = mm_ctx.__enter__()
        for w_sb, t_sb, b_sb in ((wq_sb, qt_sb, "bq"), (wk_sb, kt_sb, "bk")):
            for mo in range(KO):
                for qc in range(NQ):
                    ps = mm_ps.tile([P, 512], F32, tag="mmps")
                    for ko in range(KO):
                        nc.tensor.matmul(
                            ps[:],
                            _r(w_sb[:, ko, mo * P : (mo + 1) * P]),
                            _r(xb[:, ko, qc * 512 : (qc + 1) * 512]),
                            start=(ko == 0),
                            stop=(ko == KO - 1),
                        )
                    dst = _r(t_sb[:, mo, qc * 512 : (qc + 1) * 512])
                    if with_bias:
                        bias_t = bq_sb if b_sb == "bq" else bk_sb
                        nc.vector.tensor_scalar_add(
                            out=dst, in0=ps[:], scalar1=bias_t[:, mo]
                        )
                    else:
                        nc.vector.tensor_copy(out=dst, in_=ps[:])

        # ---- V projection: psum[so] = sum_ko x[ko,so]^T Wv[ko,:]  -> [s, nh*dv]
        for so in range(SO):
            ps = mm_ps.tile([P, 512], F32, tag="mmps")
            for ko in range(KO):
                nc.tensor.matmul(
                    ps[:],
                    _r(xb[:, ko, so * P : (so + 1) * P]),
                    _r(wv_sb[:, ko, :]),
                    start=(ko == 0),
                    stop=(ko == KO - 1),
                )
            dst = _r(v_sb[:, so, :, 0:D])  # [P, NH, D] strided (stride D+1)
            src = ps[:].rearrange("p (h d) -> p h d", h=NH)
            if with_bias:
                nc.vector.tensor_tensor(
                    dst, src, bv_rep[:].rearrange("p (h d) -> p h d", h=NH), ADD
                )
            else:
                nc.vector.tensor_copy(out=dst, in_=src)

        mm_ctx.__exit__(None, None, None)

        # ---- attention per head
        st_ctx = tc.tile_pool(name="st_ps", bufs=2, space="PSUM")
        pv_ctx = tc.tile_pool(name="pv_ps", bufs=2, space="PSUM")
        st_ps = st_ctx.__enter__()
        pv_ps = pv_ctx.__enter__()
        for h in range(NH):
            hp = h // 2
            hr = (h % 2) * D
            pv = pv_ps.tile([D + 1, S], F32, tag="pvps")
            for ki in range(SO):
                st = st_ps.tile([P, S], F32, tag="stps")
                for qc in range(NQ):
                    nc.tensor.matmul(
                        st[:, qc * 512 : (qc + 1) * 512],
                        _r(kt_sb[hr : hr + D, hp, ki * P : (ki + 1) * P]),
                        _r(qt_sb[hr : hr + D, hp, qc * 512 : (qc + 1) * 512]),
                        start=True,
                        stop=True,
                    )
                est = est_po.tile([P, S], F32, tag="est")
                nc.scalar.activation(out=_r(est[:]), in_=st[:], func=Exp, scale=1.0 / 8.0)
                for qc in range(NQ):
                    nc.tensor.matmul(
                        pv[:, qc * 512 : (qc + 1) * 512],
                        _r(v_sb[:, ki, h, :]),
                        _r(est[:, qc * 512 : (qc + 1) * 512]),
                        start=(ki == 0),
                        stop=(ki == SO - 1),
                    )
            srow = rr_po.tile([1, S], F32, tag="srow")
            nc.scalar.copy(out=srow[:], in_=pv[D : D + 1, :])
            rrow = rr_po.tile([1, S], F32, tag="rrow")
            nc.vector.reciprocal_approx_fast(out=rrow[:], in_=srow[:])
            rrep = rr_po.tile([D, S], F32, tag="rrep")
            nc.gpsimd.partition_broadcast(rrep[:], rrow[0:1, :])
            nc.vector.tensor_tensor(_r(at_sb[hr : hr + D, hp, :]), pv[0:D, :], rrep[:], MUL)

        pv_ctx.__exit__(None, None, None)
        st_ctx.__exit__(None, None, None)

        # ---- output projection + residual: outT[mo, qc] = Wo^T attnT + x_b
        mo_ctx = tc.tile_pool(name="mo_ps", bufs=2, space="PSUM")
        mm_ps = mo_ctx.__enter__()
        out_r = out.rearrange("(mo p) s -> p mo s", p=P)
        for mo in range(KO):
            for qc in range(NQ):
                ps = mm_ps.tile([P, 512], F32, tag="mmps")
                for ko in range(KO):
                    nc.tensor.matmul(
                        ps[:],
                        _r(wo_sb[:, ko, mo * P : (mo + 1) * P]),
                        _r(at_sb[:, ko, qc * 512 : (qc + 1) * 512]),
                        start=(ko == 0),
                        stop=(ko == KO - 1),
                    )
                ot = out_po.tile([P, 512], F32, tag="ot")
                nc.vector.tensor_add(
                    out=ot[:], in0=ps[:], in1=resid[:, mo, qc * 512 : (qc + 1) * 512]
                )
                nc.sync.dma_start(out=out_r[:, mo, qc * 512 : (qc + 1) * 512], in_=ot[:])
        mo_ctx.__exit__(None, None, None)

    nc.compile()
    return nc


def _get_graph(with_bias: bool):
    key = bool(with_bias)
    if key not in _GRAPH_CACHE:
        _GRAPH_CACHE[key] = _build_graph(key)
    return _GRAPH_CACHE[key]


def _make_in_maps(inputs, with_bias: bool):
    x = np.ascontiguousarray(np.asarray(inputs["x"], dtype=np.float32))
    assert x.shape == (B, C, 32, 32), x.shape
    xf = x.reshape(B, C, S)
    ws = {
        k: np.ascontiguousarray(np.asarray(inputs[k], dtype=np.float32))
        for k in ("Wq", "Wk", "Wv", "Wo")
    }
    maps = []
    for b in range(B):
        m = {"x": np.ascontiguousarray(xf[b])}
        m.update(ws)
        if with_bias:
            for k in ("bq", "bk", "bv", "bo"):
                m[k] = np.ascontiguousarray(np.asarray(inputs[k], dtype=np.float32))
        maps.append(m)
    return maps


def _run(inputs, **spmd_kwargs):
    from concourse.bass_utils import run_bass_kernel_spmd

    nh = int(np.asarray(inputs.get("num_heads", NH)))
    assert nh == NH, f"kernel hardcodes num_heads={NH}, got {nh}"
    with_bias = any(
        np.any(np.asarray(inputs[k])) for k in ("bq", "bk", "bv", "bo") if k in inputs
    )
    nc = _get_graph(with_bias)
    in_maps = _make_in_maps(inputs, with_bias)
    res = run_bass_kernel_spmd(nc, in_maps, core_ids=list(range(B)), **spmd_kwargs)
    outs = np.stack([res.results[b]["out"] for b in range(B)])  # [B, C, S]
    return outs.reshape(B, C, 32, 32).astype(np.float32), res


def kernel(**inputs):
    out, _ = _run(inputs)
    return out


# revision 10
# speedup vs baseline: 1.4096x; 1.3264x over previous
"""Multi-head self-attention (dense transformer block) on 8 TRN2 NeuronCores.

Data-parallel over batch: 8 batch items -> 8 cores, one image each, zero
collectives.  Per core the kernel computes, for x_b in [C=512, S=1024] layout
(channels x positions, which is x[b].reshape(C, H*W) -- i.e. xs^T):

  QT = Wq^T @ x_b            [nh*dk, S]   (heads on partition tiles)
  KT = Wk^T @ x_b            [nh*dk, S]
  V  = x_b^T @ Wv            [S, nh*dv]   (positions on partitions), with an
                                          appended ones-column per head
  per head h:
    st  = K_h @ Q_h^T        [S_k, S_q]   (k-positions on partitions)
    est = exp(st / 8)                     (ScalarE; no max-subtraction --
                                           scores are ~N(0,1), max ~5)
    pv  = [V_h | 1]^T @ est  [dv+1, S_q]  row dv holds sum_k est = softmax
                                           denominator (free on TensorE)
    attnT_h = pv[:dv] * (1/pv[dv])        per-q normalization
  outT = Wo^T @ attnT + x_b  [C, S]       residual; exactly the output layout

All matmuls run as float32r (fp32 storage, single-pass reduced-precision PE
mode: 1 cycle/row at N=512 vs 4 for fp32).
"""

import numpy as np

B = 8
C = 512
S = 1024
NH = 8
D = 64
P = 128
KO = C // P  # 4 partition tiles over the channel/contract dim
SO = S // P  # 8 partition tiles over positions
NQ = S // 512  # 2 free-dim chunks of 512 (fp32 moving-operand max)

_GRAPH_CACHE = {}


def _r(ap):
    """View an fp32 AP as float32r for full-rate PE matmuls."""
    import concourse.mybir as mybir

    return ap.bitcast(mybir.dt.float32r)


def _build_graph(with_bias: bool):
    import concourse.bass as bass
    import concourse.tile as tile
    from concourse import bacc, mybir
    from contextlib import ExitStack

    F32 = mybir.dt.float32
    Exp = mybir.ActivationFunctionType.Exp
    ADD = mybir.AluOpType.add
    MUL = mybir.AluOpType.mult

    nc = bacc.Bacc("TRN2", target_bir_lowering=False, debug=False, num_devices=B)

    x = nc.declare_dram_parameter("x", [C, S], F32, isOutput=False)
    wq = nc.declare_dram_parameter("Wq", [C, NH * D], F32, isOutput=False)
    wk = nc.declare_dram_parameter("Wk", [C, NH * D], F32, isOutput=False)
    wv = nc.declare_dram_parameter("Wv", [C, NH * D], F32, isOutput=False)
    wo = nc.declare_dram_parameter("Wo", [NH * D, C], F32, isOutput=False)
    if with_bias:
        bq = nc.declare_dram_parameter("bq", [NH * D], F32, isOutput=False)
        bk = nc.declare_dram_parameter("bk", [NH * D], F32, isOutput=False)
        bv = nc.declare_dram_parameter("bv", [NH * D], F32, isOutput=False)
        bo = nc.declare_dram_parameter("bo", [C], F32, isOutput=False)
    out = nc.declare_dram_parameter("out", [C, S], F32, isOutput=True)

    with ExitStack() as ctx:
        tc = ctx.enter_context(tile.TileContext(nc))
        singles = ctx.enter_context(tc.tile_pool(name="singles", bufs=1))
        est_po = ctx.enter_context(tc.tile_pool(name="est_po", bufs=10))
        out_po = ctx.enter_context(tc.tile_pool(name="out_po", bufs=3))
        rr_po = ctx.enter_context(tc.tile_pool(name="rr_po", bufs=2))

        xb = singles.tile([P, KO, S], F32, tag="xb")
        wq_sb = singles.tile([P, KO, NH * D], F32, tag="wq")
        wk_sb = singles.tile([P, KO, NH * D], F32, tag="wk")
        wv_sb = singles.tile([P, KO, NH * D], F32, tag="wv")
        wo_sb = singles.tile([P, KO, C], F32, tag="wo")
        qt_sb = singles.tile([P, KO, S], F32, tag="qt")
        kt_sb = singles.tile([P, NH, S], F32, tag="kt")  # zero-padded per head
        v_sb = singles.tile([P, SO, NH, D + 1], F32, tag="v")
        at_sb = singles.tile([P, KO, S], F32, tag="at")

        nc.sync.dma_start(out=_r(wq_sb[:]), in_=_r(wq.rearrange("(ko p) n -> p ko n", p=P)))
        nc.sync.dma_start(out=_r(xb[:]), in_=_r(x.rearrange("(ko p) s -> p ko s", p=P)))
        nc.sync.dma_start(out=_r(wk_sb[:]), in_=_r(wk.rearrange("(ko p) n -> p ko n", p=P)))
        nc.sync.dma_start(out=_r(wv_sb[:]), in_=_r(wv.rearrange("(ko p) n -> p ko n", p=P)))
        nc.sync.dma_start(out=_r(wo_sb[:]), in_=_r(wo.rearrange("(ko p) n -> p ko n", p=P)))
        zero_c = singles.tile([P, 1], F32, tag="zero")
        nc.vector.memset(zero_c[:], 0.0)
        nc.vector.tensor_copy(
            out=_r(kt_sb[:]), in_=zero_c[:].to_broadcast((P, NH, S))
        )
        ones_c = singles.tile([P, 1], F32, tag="ones")
        nc.vector.memset(ones_c[:], 1.0)
        nc.vector.tensor_copy(
            out=_r(v_sb[:, :, :, D : D + 1]),
            in_=ones_c[:].to_broadcast((P, SO, NH, 1)),
        )

        if with_bias:
            # bq/bk land on partitions (per output channel); bv along free.
            bq_sb = singles.tile([P, KO, 1], F32, tag="bq")
            bk_sb = singles.tile([P, KO, 1], F32, tag="bk")
            nc.sync.dma_start(out=bq_sb[:, :, 0], in_=bq.rearrange("(ko p) -> p ko", p=P))
            nc.sync.dma_start(out=bk_sb[:, :, 0], in_=bk.rearrange("(ko p) -> p ko", p=P))
            bv_rep = singles.tile([P, NH * D], F32, tag="bv")
            nc.sync.dma_start(
                out=bv_rep[:],
                in_=bass.AP(tensor=bv.tensor, offset=bv.offset, ap=[[0, P], [1, NH * D]]),
            )
            # xbo = x_b + bo (per-channel => per-partition scalar)
            bo_sb = singles.tile([P, KO, 1], F32, tag="bo")
            nc.sync.dma_start(out=bo_sb[:, :, 0], in_=bo.rearrange("(ko p) -> p ko", p=P))
            xbo = singles.tile([P, KO, S], F32, tag="xbo")
            for ko in range(KO):
                nc.vector.tensor_scalar_add(
                    out=xbo[:, ko, :], in0=xb[:, ko, :], scalar1=bo_sb[:, ko]
                )
            resid = xbo
        else:
            resid = xb

        # ---- QT / KT projections: psum[mo, qc] = sum_ko Wq[ko,mo]^T x[ko,qc]
        mm_ctx = tc.tile_pool(name="mm_ps", bufs=2, space="PSUM")
        mm_ps = mm_ctx.__enter__()
        for w_sb, t_sb, b_sb in ((wq_sb, qt_sb, "bq"), (wk_sb, kt_sb, "bk")):
            for mo in range(KO):
                for qc in range(NQ):
                    ps = mm_ps.tile([P, 512], F32, tag="mmps")
                    for ko in range(KO):
                        nc.tensor.matmul(
                            ps[:],
                            _r(w_sb[:, ko, mo * P : (mo + 1) * P]),
                            _r(xb[:, ko, qc * 512 : (qc + 1) * 512]),
                            start=(ko == 0),
                            stop=(ko == KO - 1),
                        )
                    if t_sb is kt_sb:
                        # per-head zero-padded layout: head 2mo at rows 0:64,
                        # head 2mo+1 at rows 64:128 of its own slot
                        for half in range(2):
                            hh = 2 * mo + half
                            hrr = half * D
                            dsth = _r(
                                kt_sb[hrr : hrr + D, hh, qc * 512 : (qc + 1) * 512]
                            )
                            if with_bias:
                                nc.vector.tensor_scalar_add(
                                    out=dsth,
                                    in0=ps[hrr : hrr + D],
                                    scalar1=bk_sb[hrr : hrr + D, mo],
                                )
                            else:
                                nc.vector.tensor_copy(
                                    out=dsth, in_=ps[hrr : hrr + D]
                                )
                        continue
                    dst = _r(t_sb[:, mo, qc * 512 : (qc + 1) * 512])
                    if with_bias:
                        nc.vector.tensor_scalar_add(
                            out=dst, in0=ps[:], scalar1=bq_sb[:, mo]
                        )
                    else:
                        nc.vector.tensor_copy(out=dst, in_=ps[:])

        # ---- V projection: psum[so] = sum_ko x[ko,so]^T Wv[ko,:]  -> [s, nh*dv]
        for so in range(SO):
            ps = mm_ps.tile([P, 512], F32, tag="mmps")
            for ko in range(KO):
                nc.tensor.matmul(
                    ps[:],
                    _r(xb[:, ko, so * P : (so + 1) * P]),
                    _r(wv_sb[:, ko, :]),
                    start=(ko == 0),
                    stop=(ko == KO - 1),
                )
            dst = _r(v_sb[:, so, :, 0:D])  # [P, NH, D] strided (stride D+1)
            src = ps[:].rearrange("p (h d) -> p h d", h=NH)
            if with_bias:
                nc.vector.tensor_tensor(
                    dst, src, bv_rep[:].rearrange("p (h d) -> p h d", h=NH), ADD
                )
            else:
                nc.vector.tensor_copy(out=dst, in_=src)

        mm_ctx.__exit__(None, None, None)

        # ---- attention per head
        st_ctx = tc.tile_pool(name="st_ps", bufs=2, space="PSUM")
        pv_ctx = tc.tile_pool(name="pv_ps", bufs=2, space="PSUM")
        st_ps = st_ctx.__enter__()
        pv_ps = pv_ctx.__enter__()
        for h in range(NH):
            hp = h // 2
            hr = (h % 2) * D
            pv = pv_ps.tile([D + 1, S], F32, tag="pvps")
            for ki in range(SO):
                st = st_ps.tile([P, S], F32, tag="stps")
                for qc in range(NQ):
                    nc.tensor.matmul(
                        st[:, qc * 512 : (qc + 1) * 512],
                        _r(kt_sb[:, h, ki * P : (ki + 1) * P]),
                        _r(qt_sb[:, hp, qc * 512 : (qc + 1) * 512]),
                        start=True,
                        stop=True,
                    )
                est = est_po.tile([P, S], F32, tag="est")
                nc.scalar.activation(out=_r(est[:]), in_=st[:], func=Exp, scale=1.0 / 8.0)
                for qc in range(NQ):
                    nc.tensor.matmul(
                        pv[:, qc * 512 : (qc + 1) * 512],
                        _r(v_sb[:, ki, h, :]),
                        _r(est[:, qc * 512 : (qc + 1) * 512]),
                        start=(ki == 0),
                        stop=(ki == SO - 1),
                    )
            srow = rr_po.tile([1, S], F32, tag="srow")
            nc.scalar.copy(out=srow[:], in_=pv[D : D + 1, :])
            rrow = rr_po.tile([1, S], F32, tag="rrow")
            nc.vector.reciprocal_approx_fast(out=rrow[:], in_=srow[:])
            rrep = rr_po.tile([D, S], F32, tag="rrep")
            nc.gpsimd.partition_broadcast(rrep[:], rrow[0:1, :])
            nc.vector.tensor_tensor(_r(at_sb[hr : hr + D, hp, :]), pv[0:D, :], rrep[:], MUL)

        pv_ctx.__exit__(None, None, None)
        st_ctx.__exit__(None, None, None)

        # ---- output projection + residual: outT[mo, qc] = Wo^T attnT + x_b
        mo_ctx = tc.tile_pool(name="mo_ps", bufs=2, space="PSUM")
        mm_ps = mo_ctx.__enter__()
        out_r = out.rearrange("(mo p) s -> p mo s", p=P)
        for mo in range(KO):
            for qc in range(NQ):
                ps = mm_ps.tile([P, 512], F32, tag="mmps")
                for ko in range(KO):
                    nc.tensor.matmul(
                        ps[:],
                        _r(wo_sb[:, ko, mo * P : (mo + 1) * P]),
                        _r(at_sb[:, ko, qc * 512 : (qc + 1) * 512]),
                        start=(ko == 0),
                        stop=(ko == KO - 1),
                    )
                ot = out_po.tile([P, 512], F32, tag="ot")
                nc.vector.tensor_add(
                    out=ot[:], in0=ps[:], in1=resid[:, mo, qc * 512 : (qc + 1) * 512]
                )
                nc.sync.dma_start(out=out_r[:, mo, qc * 512 : (qc + 1) * 512], in_=ot[:])
        mo_ctx.__exit__(None, None, None)

    nc.compile()
    return nc


def _get_graph(with_bias: bool):
    key = bool(with_bias)
    if key not in _GRAPH_CACHE:
        _GRAPH_CACHE[key] = _build_graph(key)
    return _GRAPH_CACHE[key]


def _make_in_maps(inputs, with_bias: bool):
    x = np.ascontiguousarray(np.asarray(inputs["x"], dtype=np.float32))
    assert x.shape == (B, C, 32, 32), x.shape
    xf = x.reshape(B, C, S)
    ws = {
        k: np.ascontiguousarray(np.asarray(inputs[k], dtype=np.float32))
        for k in ("Wq", "Wk", "Wv", "Wo")
    }
    maps = []
    for b in range(B):
        m = {"x": np.ascontiguousarray(xf[b])}
        m.update(ws)
        if with_bias:
            for k in ("bq", "bk", "bv", "bo"):
                m[k] = np.ascontiguousarray(np.asarray(inputs[k], dtype=np.float32))
        maps.append(m)
    return maps


def _run(inputs, **spmd_kwargs):
    from concourse.bass_utils import run_bass_kernel_spmd

    nh = int(np.asarray(inputs.get("num_heads", NH)))
    assert nh == NH, f"kernel hardcodes num_heads={NH}, got {nh}"
    with_bias = any(
        np.any(np.asarray(inputs[k])) for k in ("bq", "bk", "bv", "bo") if k in inputs
    )
    nc = _get_graph(with_bias)
    in_maps = _make_in_maps(inputs, with_bias)
    res = run_bass_kernel_spmd(nc, in_maps, core_ids=list(range(B)), **spmd_kwargs)
    outs = np.stack([res.results[b]["out"] for b in range(B)])  # [B, C, S]
    return outs.reshape(B, C, 32, 32).astype(np.float32), res


def kernel(**inputs):
    out, _ = _run(inputs)
    return out


# revision 11
# speedup vs baseline: 1.5354x; 1.0892x over previous
"""Multi-head self-attention (dense transformer block) on 8 TRN2 NeuronCores.

Data-parallel over batch: 8 batch items -> 8 cores, one image each, zero
collectives.  Per core the kernel computes, for x_b in [C=512, S=1024] layout
(channels x positions, which is x[b].reshape(C, H*W) -- i.e. xs^T):

  QT = Wq^T @ x_b            [nh*dk, S]   (heads on partition tiles)
  KT = Wk^T @ x_b            [nh*dk, S]   (stored zero-padded per head so the
                                           scores matmul contracts over a full
                                           128 rows -- half-array K=64 matmuls
                                           don't register as busy to the PE
                                           clock gate (HAM) and run at 1.2GHz)
  V  = x_b^T @ Wv            [S, nh*dv]   (positions on partitions), with an
                                          appended ones-column per head
  per head h:
    st  = K_h @ Q_h^T        [S_k, S_q]   (k-positions on partitions)
    est = exp(st / 8)                     (ScalarE; no max-subtraction --
                                           scores stay within +-32, exp is
                                           comfortably inside fp32 range)
    pv  = [V_h | 1]^T @ est  [dv+1, S_q]  row dv holds sum_k est = softmax
                                           denominator (free on TensorE)
    attnT_h = pv[:dv] * (1/pv[dv])        per-q normalization
  outT = Wo^T @ attnT + x_b  [C, S]       residual; exactly the output layout

All matmul operands are bf16 (fp32 PSUM accumulation): 1 cycle/row streaming
(fp32/fp32r stream 4-byte operands at ~2 cycles/row) plus fast weight load.
The residual add uses the exact fp32 x.
"""

import numpy as np

B = 8
C = 512
S = 1024
NH = 8
D = 64
P = 128
KO = C // P  # 4 partition tiles over the channel/contract dim
SO = S // P  # 8 partition tiles over positions
NQ = S // 512  # 2 free-dim chunks of 512 per matmul (PSUM bank limit)

_GRAPH_CACHE = {}


def _build_graph(with_bias: bool):
    import concourse.bass as bass
    import concourse.tile as tile
    from concourse import bacc, mybir
    from contextlib import ExitStack

    F32 = mybir.dt.float32
    BF16 = mybir.dt.bfloat16
    Exp = mybir.ActivationFunctionType.Exp
    ADD = mybir.AluOpType.add
    MUL = mybir.AluOpType.mult

    nc = bacc.Bacc("TRN2", target_bir_lowering=False, debug=False, num_devices=B)

    x = nc.declare_dram_parameter("x", [C, S], F32, isOutput=False)
    wq = nc.declare_dram_parameter("Wq", [C, NH * D], F32, isOutput=False)
    wk = nc.declare_dram_parameter("Wk", [C, NH * D], F32, isOutput=False)
    wv = nc.declare_dram_parameter("Wv", [C, NH * D], F32, isOutput=False)
    wo = nc.declare_dram_parameter("Wo", [NH * D, C], F32, isOutput=False)
    if with_bias:
        bq = nc.declare_dram_parameter("bq", [NH * D], F32, isOutput=False)
        bk = nc.declare_dram_parameter("bk", [NH * D], F32, isOutput=False)
        bv = nc.declare_dram_parameter("bv", [NH * D], F32, isOutput=False)
        bo = nc.declare_dram_parameter("bo", [C], F32, isOutput=False)
    out = nc.declare_dram_parameter("out", [C, S], F32, isOutput=True)

    with ExitStack() as ctx:
        tc = ctx.enter_context(tile.TileContext(nc))
        singles = ctx.enter_context(tc.tile_pool(name="singles", bufs=1))
        est_po = ctx.enter_context(tc.tile_pool(name="est_po", bufs=12))
        out_po = ctx.enter_context(tc.tile_pool(name="out_po", bufs=3))
        rr_po = ctx.enter_context(tc.tile_pool(name="rr_po", bufs=2))

        xb = singles.tile([P, KO, S], F32, tag="xb")
        xb_bf = singles.tile([P, KO, S], BF16, tag="xbb")
        wq_sb = singles.tile([P, KO, NH * D], BF16, tag="wq")
        wk_sb = singles.tile([P, KO, NH * D], BF16, tag="wk")
        wv_sb = singles.tile([P, KO, NH * D], BF16, tag="wv")
        wo_sb = singles.tile([P, KO, C], BF16, tag="wo")
        qt_sb = singles.tile([P, KO, S], BF16, tag="qt")
        kt_sb = singles.tile([P, NH, S], BF16, tag="kt")  # zero-padded per head
        v_sb = singles.tile([P, SO, NH, D + 1], BF16, tag="v")
        at_sb = singles.tile([P, KO, S], BF16, tag="at")

        # x loaded as exact fp32 (kept for the residual), weights cast-loaded
        # to bf16 by the software-DGE (gpsimd) DMA path.
        nc.gpsimd.dma_start(out=wq_sb[:], in_=wq.rearrange("(ko p) n -> p ko n", p=P))
        nc.sync.dma_start(out=xb[:], in_=x.rearrange("(ko p) s -> p ko s", p=P))
        nc.gpsimd.dma_start(out=wk_sb[:], in_=wk.rearrange("(ko p) n -> p ko n", p=P))
        nc.gpsimd.dma_start(out=wv_sb[:], in_=wv.rearrange("(ko p) n -> p ko n", p=P))
        nc.gpsimd.dma_start(out=wo_sb[:], in_=wo.rearrange("(ko p) n -> p ko n", p=P))
        for ko in range(KO):
            nc.vector.tensor_copy(out=xb_bf[:, ko, :], in_=xb[:, ko, :])

        zero_c = singles.tile([P, 1], F32, tag="zero")
        nc.vector.memset(zero_c[:], 0.0)
        nc.vector.tensor_copy(out=kt_sb[:], in_=zero_c[:].to_broadcast((P, NH, S)))
        ones_c = singles.tile([P, 1], F32, tag="ones")
        nc.vector.memset(ones_c[:], 1.0)
        nc.vector.tensor_copy(
            out=v_sb[:, :, :, D : D + 1],
            in_=ones_c[:].to_broadcast((P, SO, NH, 1)),
        )

        if with_bias:
            # bq/bk land on partitions (per output channel); bv along free.
            bq_sb = singles.tile([P, KO, 1], F32, tag="bq")
            bk_sb = singles.tile([P, KO, 1], F32, tag="bk")
            nc.sync.dma_start(out=bq_sb[:, :, 0], in_=bq.rearrange("(ko p) -> p ko", p=P))
            nc.sync.dma_start(out=bk_sb[:, :, 0], in_=bk.rearrange("(ko p) -> p ko", p=P))
            bv_rep = singles.tile([P, NH * D], F32, tag="bv")
            nc.sync.dma_start(
                out=bv_rep[:],
                in_=bass.AP(tensor=bv.tensor, offset=bv.offset, ap=[[0, P], [1, NH * D]]),
            )
            # xbo = x_b + bo (per-channel => per-partition scalar)
            bo_sb = singles.tile([P, KO, 1], F32, tag="bo")
            nc.sync.dma_start(out=bo_sb[:, :, 0], in_=bo.rearrange("(ko p) -> p ko", p=P))
            xbo = singles.tile([P, KO, S], F32, tag="xbo")
            for ko in range(KO):
                nc.vector.tensor_scalar_add(
                    out=xbo[:, ko, :], in0=xb[:, ko, :], scalar1=bo_sb[:, ko]
                )
            resid = xbo
        else:
            resid = xb

        # ---- QT / KT projections: psum[mo, qc] = sum_ko Wq[ko,mo]^T x[ko,qc]
        mm_ctx = tc.tile_pool(name="mm_ps", bufs=2, space="PSUM")
        mm_ps = mm_ctx.__enter__()
        for w_sb, t_sb in ((wq_sb, qt_sb), (wk_sb, kt_sb)):
            for mo in range(KO):
                for qc in range(NQ):
                    ps = mm_ps.tile([P, 512], F32, tag="mmps")
                    for ko in range(KO):
                        nc.tensor.matmul(
                            ps[:],
                            w_sb[:, ko, mo * P : (mo + 1) * P],
                            xb_bf[:, ko, qc * 512 : (qc + 1) * 512],
                            start=(ko == 0),
                            stop=(ko == KO - 1),
                        )
                    if t_sb is kt_sb:
                        # per-head zero-padded layout: head 2mo at rows 0:64,
                        # head 2mo+1 at rows 64:128 of its own slot
                        for half in range(2):
                            hh = 2 * mo + half
                            hrr = half * D
                            dsth = kt_sb[hrr : hrr + D, hh, qc * 512 : (qc + 1) * 512]
                            if with_bias:
                                nc.vector.tensor_scalar_add(
                                    out=dsth,
                                    in0=ps[hrr : hrr + D],
                                    scalar1=bk_sb[hrr : hrr + D, mo],
                                )
                            else:
                                nc.vector.tensor_copy(out=dsth, in_=ps[hrr : hrr + D])
                        continue
                    dst = t_sb[:, mo, qc * 512 : (qc + 1) * 512]
                    if with_bias:
                        nc.vector.tensor_scalar_add(
                            out=dst, in0=ps[:], scalar1=bq_sb[:, mo]
                        )
                    else:
                        nc.vector.tensor_copy(out=dst, in_=ps[:])

        # ---- V projection: psum[so] = sum_ko x[ko,so]^T Wv[ko,:]  -> [s, nh*dv]
        for so in range(SO):
            ps = mm_ps.tile([P, 512], F32, tag="mmps")
            for ko in range(KO):
                nc.tensor.matmul(
                    ps[:],
                    xb_bf[:, ko, so * P : (so + 1) * P],
                    wv_sb[:, ko, :],
                    start=(ko == 0),
                    stop=(ko == KO - 1),
                )
            dst = v_sb[:, so, :, 0:D]  # [P, NH, D] strided (stride D+1)
            src = ps[:].rearrange("p (h d) -> p h d", h=NH)
            if with_bias:
                nc.vector.tensor_tensor(
                    dst, src, bv_rep[:].rearrange("p (h d) -> p h d", h=NH), ADD
                )
            else:
                nc.vector.tensor_copy(out=dst, in_=src)

        mm_ctx.__exit__(None, None, None)

        # ---- attention per head
        st_ctx = tc.tile_pool(name="st_ps", bufs=2, space="PSUM")
        pv_ctx = tc.tile_pool(name="pv_ps", bufs=2, space="PSUM")
        st_ps = st_ctx.__enter__()
        pv_ps = pv_ctx.__enter__()
        for h in range(NH):
            hp = h // 2
            hr = (h % 2) * D
            pv = pv_ps.tile([D + 1, S], F32, tag="pvps")
            for ki in range(SO):
                st = st_ps.tile([P, S], F32, tag="stps")
                for qc in range(NQ):
                    nc.tensor.matmul(
                        st[:, qc * 512 : (qc + 1) * 512],
                        kt_sb[:, h, ki * P : (ki + 1) * P],
                        qt_sb[:, hp, qc * 512 : (qc + 1) * 512],
                        start=True,
                        stop=True,
                    )
                est = est_po.tile([P, S], BF16, tag="est")
                nc.scalar.activation(out=est[:], in_=st[:], func=Exp, scale=1.0 / 8.0)
                for qc in range(NQ):
                    nc.tensor.matmul(
                        pv[:, qc * 512 : (qc + 1) * 512],
                        v_sb[:, ki, h, :],
                        est[:, qc * 512 : (qc + 1) * 512],
                        start=(ki == 0),
                        stop=(ki == SO - 1),
                    )
            srow = rr_po.tile([1, S], F32, tag="srow")
            nc.vector.tensor_copy(out=srow[:], in_=pv[D : D + 1, :])
            rrow = rr_po.tile([1, S], F32, tag="rrow")
            nc.vector.reciprocal_approx_fast(out=rrow[:], in_=srow[:])
            rrep = rr_po.tile([D, S], F32, tag="rrep")
            nc.gpsimd.partition_broadcast(rrep[:], rrow[0:1, :])
            nc.vector.tensor_tensor(at_sb[hr : hr + D, hp, :], pv[0:D, :], rrep[:], MUL)
        pv_ctx.__exit__(None, None, None)
        st_ctx.__exit__(None, None, None)

        # ---- output projection + residual: outT[mo, qc] = Wo^T attnT + x_b
        mo_ctx = tc.tile_pool(name="mo_ps", bufs=2, space="PSUM")
        mm_ps = mo_ctx.__enter__()
        out_r = out.rearrange("(mo p) s -> p mo s", p=P)
        for mo in range(KO):
            for qc in range(NQ):
                ps = mm_ps.tile([P, 512], F32, tag="mops")
                for ko in range(KO):
                    nc.tensor.matmul(
                        ps[:],
                        wo_sb[:, ko, mo * P : (mo + 1) * P],
                        at_sb[:, ko, qc * 512 : (qc + 1) * 512],
                        start=(ko == 0),
                        stop=(ko == KO - 1),
                    )
                ot = out_po.tile([P, 512], F32, tag="ot")
                nc.vector.tensor_add(
                    out=ot[:], in0=ps[:], in1=resid[:, mo, qc * 512 : (qc + 1) * 512]
                )
                nc.sync.dma_start(out=out_r[:, mo, qc * 512 : (qc + 1) * 512], in_=ot[:])
        mo_ctx.__exit__(None, None, None)

    nc.compile()
    return nc


def _get_graph(with_bias: bool):
    key = bool(with_bias)
    if key not in _GRAPH_CACHE:
        _GRAPH_CACHE[key] = _build_graph(key)
    return _GRAPH_CACHE[key]


def _make_in_maps(inputs, with_bias: bool):
    x = np.ascontiguousarray(np.asarray(inputs["x"], dtype=np.float32))
    assert x.shape == (B, C, 32, 32), x.shape
    xf = x.reshape(B, C, S)
    ws = {
        k: np.ascontiguousarray(np.asarray(inputs[k], dtype=np.float32))
        for k in ("Wq", "Wk", "Wv", "Wo")
    }
    maps = []
    for b in range(B):
        m = {"x": np.ascontiguousarray(xf[b])}
        m.update(ws)
        if with_bias:
            for k in ("bq", "bk", "bv", "bo"):
                m[k] = np.ascontiguousarray(np.asarray(inputs[k], dtype=np.float32))
        maps.append(m)
    return maps


def _run(inputs, **spmd_kwargs):
    from concourse.bass_utils import run_bass_kernel_spmd

    nh = int(np.asarray(inputs.get("num_heads", NH)))
    assert nh == NH, f"kernel hardcodes num_heads={NH}, got {nh}"
    with_bias = any(
        np.any(np.asarray(inputs[k])) for k in ("bq", "bk", "bv", "bo") if k in inputs
    )
    nc = _get_graph(with_bias)
    in_maps = _make_in_maps(inputs, with_bias)
    res = run_bass_kernel_spmd(nc, in_maps, core_ids=list(range(B)), **spmd_kwargs)
    outs = np.stack([res.results[b]["out"] for b in range(B)])  # [B, C, S]
    return outs.reshape(B, C, 32, 32).astype(np.float32), res


def kernel(**inputs):
    out, _ = _run(inputs)
    return out
